# revision 39
# baseline (speedup 1.0000x reference)
"""Trainium2 Bass kernel for MultiHeadLatentAttention (B=2, S=2048, H=2048,
NH=16, HD=128, LAT=512), SPMD across 8 NeuronCores.

Sharding: 8 cores = 2 (batch) x 4 (head-group TP). Core c handles batch c//4
and head group j = c%4 = heads {j, 4+j, 8+j, 12+j}. That grouping is chosen so
the 4 heads share exactly 256 rows of Wq_up/Wk_up: heads j and 4+j are the raw
x1/x2 slices of q_half, heads 8+j and 12+j are their RoPE combinations - so
the up-projection shards 4-way with no duplication. Each core computes its
partial o_proj output; the host sums the 4 partials per batch and adds bo.

v2: fp8(e4m3) DoubleRow matmuls (2 k-tiles per instruction = 2x PE rate) for
the fused-q projection and the attention AV contraction; exp probabilities are
written as fp8 by the ACT exp; softmax denominator is a 4-instruction balanced
strided tree on DVE. kv_down optionally fp8 via KV_FP8.

Self-contained: builds + compiles the Bass program on first call (cached),
runs via run_bass_kernel_spmd on cores 0-7.
"""
import os
import sys
import types
from contextlib import ExitStack

import numpy as np

if "/opt/trn_rl_repo" not in sys.path:
    sys.path.insert(0, "/opt/trn_rl_repo")

import ml_dtypes

# ---------------------------------------------------------------------------
# NTFF-profile shim: antenv.axon_hooks is missing in this image; register a
# hook backed by the axon PJRT .so so trace=True can capture HW exec time.
# ---------------------------------------------------------------------------


def _install_axon_hooks_shim():
    if "antenv.axon_hooks" in sys.modules:
        return
    try:
        import antenv
        from trn_agent_boot.trn_boot import _ntff_profile_via_ctypes
        hook = _ntff_profile_via_ctypes("/opt/axon/libaxon_pjrt.so")
    except Exception:
        return
    mod = types.ModuleType("antenv.axon_hooks")
    mod.get_axon_ntff_profile_hook = lambda: hook
    mod.set_axon_ntff_profile_hook = lambda h: None
    sys.modules["antenv.axon_hooks"] = mod
    antenv.axon_hooks = mod


_install_axon_hooks_shim()

import concourse.bass as bass  # noqa: E402
import concourse.mybir as mybir  # noqa: E402
import concourse.tile as tile  # noqa: E402
from concourse import bacc  # noqa: E402
from concourse.bass_utils import run_bass_kernel_spmd  # noqa: E402

P = 128
H = 2048
NH = 16
HD = 128
LAT = 512
B = 2
S = 2048
ROPE_DIM = H // 4
NHG = 4          # heads per core
SC = 512         # s/q chunk (one PSUM bank of fp32)
INV_SQRT_HD = 0.08838834764831845  # 1/sqrt(128)

# fp8 pre-scales (host side, folded back out via ACT scale)
S_HS = 32.0       # hidden_states: absmax ~5.3 -> 170 < 240
S_WQE = 1024.0    # fused q weight: absmax ~0.05 -> 51
S_WKVD = 512.0    # kv_down weight: absmax ~0.1 -> 51
KV_FP8 = bool(int(os.environ.get("MLA_KV_FP8", "1")))

f32 = mybir.dt.float32
f32r = mybir.dt.float32r
bf16 = mybir.dt.bfloat16
f16 = mybir.dt.float16
f8 = mybir.dt.float8e4
Act = mybir.ActivationFunctionType
Alu = mybir.AluOpType
DR = mybir.MatmulPerfMode.DoubleRow
BF16 = ml_dtypes.bfloat16
F8 = ml_dtypes.float8_e4m3
F16 = np.float16


def build_mla(seq=S, kv_fp8=KV_FP8, debug=False):
    """Build one core's program. All cores run this same program SPMD."""
    NSC = seq // SC   # s-chunks
    HT = H // P       # 16 h-tiles
    LT = LAT // P     # 4 l-tiles
    ST = seq // P     # s-tiles (= k-tiles in attention)

    nc = bacc.Bacc("TRN2", target_bir_lowering=False, debug=debug)

    # all inputs host-swizzled to partition-major [P, nt, free] so each is
    # one DMA of long contiguous per-partition rows (the naive [H, S]
    # layouts load as 512B descriptors and are descriptor-rate-bound)
    hs8 = nc.dram_tensor("hs8", [seq // SC, P, H // P, SC], f8,
                         kind="ExternalInput")
    assert kv_fp8, "f16 kv_down path removed in v3.3"
    Wkvd8 = nc.dram_tensor("Wkvd8", [LAT // P, P, H // P, P], f8,
                           kind="ExternalInput")
    Wqe8 = nc.dram_tensor("Wqe8", [2, P, H // P, P], f8,
                          kind="ExternalInput")
    bkvd = nc.dram_tensor("bkvd", [LAT], f32, kind="ExternalInput")
    Wku = nc.dram_tensor("Wku", [P, LAT // P, 2 * P], f16,
                         kind="ExternalInput")
    bqku = nc.dram_tensor("bqku", [P, 4], f32, kind="ExternalInput")
    Wvu = nc.dram_tensor("Wvu", [P, LAT // P, NHG * P], f16,
                         kind="ExternalInput")
    bvu = nc.dram_tensor("bvu", [1, NHG * P], f32, kind="ExternalInput")
    Wo = nc.dram_tensor("Wo", [P, NHG, H], f16, kind="ExternalInput")
    cosT = nc.dram_tensor("cosT", [P, seq], f16, kind="ExternalInput")
    sinT = nc.dram_tensor("sinT", [P, seq], f16, kind="ExternalInput")
    outT = nc.dram_tensor("outT", [H, seq], f16, kind="ExternalOutput")

    with tile.TileContext(nc) as tc, ExitStack() as top:
        const = top.enter_context(tc.tile_pool(name="const", bufs=1))
        ao_pool = top.enter_context(tc.tile_pool(name="ao", bufs=1))

        bkvd_t = const.tile([P, LT], f32)
        nc.sync.dma_start(bkvd_t[:], bkvd.rearrange("(o p) -> p o", p=P))
        # on-chip ones (fp8 so it can be the DoubleRow stationary of the
        # softmax-denominator reduction): no DMA dependency, so the HAM
        # warmup starts as soon as the DVE is live
        ones8 = const.tile([P, 2, P], f8)
        nc.vector.memset(ones8[:], 1.0)
        ones16 = const.tile([P, P], f16)
        nc.vector.memset(ones16[:], 1.0)

        # HAM warmup: back-to-back matmuls (~3.5us of PE activity) while
        # the initial weight/activation DMAs stream in, so the first real
        # matmuls run at 2.4GHz instead of the cold 1.2GHz.
        with tc.tile_pool(name="warm", bufs=1, space="PSUM") as warm_pool:
            wtiles = [warm_pool.tile([P, P], f32, tag=f"w{i}", name=f"warm{i}")
                      for i in range(4)]
            for i in range(96):
                nc.tensor.matmul(wtiles[i % 4][:], ones8[:, 0, :],
                                 ones8[:, 1, :], start=True, stop=True)

        attn_outT = ao_pool.tile([P, NHG, seq], f16)

        with ExitStack() as qkv_scope:
            qk_pool = qkv_scope.enter_context(tc.tile_pool(name="qk", bufs=1))
            v_pool = qkv_scope.enter_context(tc.tile_pool(name="v", bufs=1))
            qT = qk_pool.tile([P, NHG, seq], f16)  # 0=x1, 1=x2, 2,3=rope
            kT = qk_pool.tile([P, NHG, seq], f16)
            v8 = v_pool.tile([P, ST, NHG * P], f8)  # token-major v, fp8

            with ExitStack() as lat_scope:
                lat_pool = lat_scope.enter_context(
                    tc.tile_pool(name="lat", bufs=1))
                kv_latT = lat_pool.tile([P, LT, seq], f16)
                # early-U pool + U psum hoisted ABOVE the D pools: v-up can
                # start the moment the last D chain drains, instead of
                # waiting for D's pools to release and ~2.5MB of U DMAs.
                ue_pool = lat_scope.enter_context(
                    tc.tile_pool(name="uearly", bufs=1))
                psu = lat_scope.enter_context(
                    tc.tile_pool(name="psu", bufs=4, space="PSUM"))
                bqku_t = ue_pool.tile([P, 4], f32)
                bvu_bc = ue_pool.tile([P, NHG * P], f32)
                wvu_t = ue_pool.tile([P, LT, NHG * P], f16)
                cos_t = ue_pool.tile([P, seq], f16)
                sin_t = ue_pool.tile([P, seq], f16)
                wku_t = ue_pool.tile([P, LT, 2 * P], f16)
                ut_pool = lat_scope.enter_context(
                    tc.tile_pool(name="ut", bufs=4))

                # ---------------- phase D: projections from hs -------------
                # per s-chunk: 4 kv_lat chains + 2 fused-q chains. The q
                # down+up pair is algebraically folded into one [H, 256]
                # effective weight (Wqd.T @ Wqu_sel.T) on the host, so the
                # duplicated q-down never runs on-device. The fused-q (and
                # optionally kv) chains run as fp8 DoubleRow: 8 instructions
                # contract all 16 h-tiles.
                with tc.tile_pool(name="wd", bufs=1) as wd_pool, \
                     tc.tile_pool(name="psd", bufs=4, space="PSUM") as psd:
                    # load order: activations first (chains need all 16
                    # h-tiles), then Wkv_down (chains m=0..3), then Wqe
                    h8 = wd_pool.tile([P, HT, seq], f8)
                    wkvd8_t = wd_pool.tile([P, HT, LAT], f8)
                    wqe8_t = wd_pool.tile([P, HT, 2 * P], f8)
                    # issue order = first-chain critical path first: chunk 0
                    # activations + the Wkv_down quarters, then everything
                    # else (chunks 1-3 are only needed ~12us later)
                    nc.sync.dma_start(h8[:, :, 0:SC], hs8[0])
                    for m in range(LT):
                        nc.sync.dma_start(
                            wkvd8_t[:, :, m * P:(m + 1) * P], Wkvd8[m])
                    for ci in range(2):
                        nc.sync.dma_start(
                            wqe8_t[:, :, ci * P:(ci + 1) * P], Wqe8[ci])
                    for sc in range(1, NSC):
                        nc.sync.dma_start(
                            h8[:, :, sc * SC:(sc + 1) * SC], hs8[sc])
                    nc.sync.dma_start(bqku_t[:], bqku[:])
                    nc.sync.dma_start(
                        bvu_bc[:], bvu[:].to_broadcast((P, NHG * P)))
                    nc.sync.dma_start(wvu_t[:], Wvu[:])
                    nc.sync.dma_start(wku_t[:], Wku[:])
                    nc.sync.dma_start(cos_t[:], cosT[:])
                    nc.sync.dma_start(sin_t[:], sinT[:])

                    def rope(dstT, ssl):
                        # rope: slot2 = x1*cos - x2*sin,
                        #       slot3 = x1*sin + x2*cos
                        x1 = dstT[:, 0, ssl]
                        x2 = dstT[:, 1, ssl]
                        t1 = ut_pool.tile([P, SC], f16, tag="ropetmp")
                        t2 = ut_pool.tile([P, SC], f16, tag="ropetmp")
                        nc.vector.tensor_mul(t1[:], x1, cos_t[:, ssl])
                        nc.vector.tensor_mul(t2[:], x2, sin_t[:, ssl])
                        nc.vector.tensor_sub(dstT[:, 2, ssl], t1[:], t2[:])
                        t3 = ut_pool.tile([P, SC], f16, tag="ropetmp")
                        t4 = ut_pool.tile([P, SC], f16, tag="ropetmp")
                        nc.vector.tensor_mul(t3[:], x1, sin_t[:, ssl])
                        nc.vector.tensor_mul(t4[:], x2, cos_t[:, ssl])
                        nc.vector.tensor_add(dstT[:, 3, ssl], t3[:], t4[:])

                    # emission order per chunk: kv chains, k_up, k-rope,
                    # q chains, q-rope, v_up - every DVE op lands with PE
                    # matmul work after it, so the k/q tensors a chunk
                    # contributes are ready the moment its PE work drains
                    # (no DVE tail at the D->attention boundary).
                    for sc in range(NSC):
                        ssl = slice(sc * SC, (sc + 1) * SC)
                        for m in range(LT):
                            ps = psd.tile([P, SC], f32)
                            w8 = wkvd8_t[:, :, m * P:(m + 1) * P]
                            for t in range(HT // 2):
                                nc.tensor.matmul(
                                    ps[:], w8[:, 2 * t:2 * t + 2, :],
                                    h8[:, 2 * t:2 * t + 2, ssl],
                                    start=(t == 0),
                                    stop=(t == HT // 2 - 1),
                                    perf_mode=DR)
                            nc.scalar.activation(
                                kv_latT[:, m, ssl], ps[:], Act.Identity,
                                bias=bkvd_t[:, m:m + 1],
                                scale=1.0 / (S_HS * S_WKVD))

                        for ci in (2, 3):  # k_x1, k_x2
                            csl = slice((ci % 2) * P, (ci % 2) * P + P)
                            ps = psu.tile([P, SC], f32, tag="psu")
                            for lt in range(LT):
                                nc.tensor.matmul(
                                    ps[:], wku_t[:, lt, csl],
                                    kv_latT[:, lt, ssl],
                                    start=(lt == 0), stop=(lt == LT - 1))
                            # bias-add on DVE (free-dim broadcast of [P,1])
                            # to keep ACT free for the attention exps
                            nc.vector.tensor_tensor(
                                kT[:, ci % 2, ssl], ps[:],
                                bqku_t[:, ci:ci + 1].to_broadcast((P, SC)),
                                Alu.add)
                        rope(kT, ssl)

                        for ci in range(2):
                            ps = psd.tile([P, SC], f32)
                            wq8 = wqe8_t[:, :, ci * P:(ci + 1) * P]
                            for t in range(HT // 2):
                                nc.tensor.matmul(
                                    ps[:], wq8[:, 2 * t:2 * t + 2, :],
                                    h8[:, 2 * t:2 * t + 2, ssl],
                                    start=(t == 0),
                                    stop=(t == HT // 2 - 1),
                                    perf_mode=DR)
                            # fused q: bias col ci of bqku = b_eff slice
                            nc.scalar.activation(
                                qT[:, ci, ssl], ps[:], Act.Identity,
                                bias=bqku_t[:, ci:ci + 1],
                                scale=1.0 / (S_HS * S_WQE))
                        rope(qT, ssl)

                        for stl in range(SC // P):
                            st = sc * (SC // P) + stl
                            ps = psu.tile([P, NHG * P], f32, tag="psu")
                            for lt in range(LT):
                                nc.tensor.matmul(
                                    ps[:],
                                    kv_latT[:, lt, st * P:(st + 1) * P],
                                    wvu_t[:, lt, :],
                                    start=(lt == 0), stop=(lt == LT - 1))
                            nc.vector.tensor_tensor(
                                v8[:, st, :], ps[:], bvu_bc[:], Alu.add)

            # ------- phase A+O: attention with o_proj interleaved --------
            # qc-outer: once all 4 heads of a q-chunk are normalized, that
            # chunk's o_proj runs on PE underneath the next chunk's
            # ACT-bound score/exp pipeline.
            with tc.tile_pool(name="exp", bufs=3) as exp_pool, \
                 tc.tile_pool(name="wo", bufs=1) as wo_pool, \
                 tc.tile_pool(name="pss", bufs=2, space="PSUM") as pss, \
                 tc.tile_pool(name="psav", bufs=1, space="PSUM") as psav, \
                 tc.tile_pool(name="pssm", bufs=1, space="PSUM") as pssm, \
                 tc.tile_pool(name="pso", bufs=2, space="PSUM") as pso, \
                 tc.tile_pool(name="att", bufs=3) as at_pool, \
                 tc.tile_pool(name="ot", bufs=4) as ot_pool:
                # o_proj weights resident: [c-part, ct, m] layout
                wo_t = wo_pool.tile([P, NHG, H], f16)
                nc.sync.dma_start(wo_t[:], Wo[:])

                def scores_exp(qc, h):
                    qsl = slice(qc * SC, (qc + 1) * SC)
                    expt = exp_pool.tile([P, ST, SC], f8, tag="expt",
                                         name="expt")
                    # k-tiles in pairs: two matmuls fill a 2-bank psum
                    # tile, one ACT exp covers both (amortizes the ~240ns
                    # per-ACT-instruction overhead) and writes fp8 probs.
                    for kth in range(ST // 2):
                        ps = pss.tile([P, 2, SC], f32, tag="score",
                                      name="score")
                        for half in (0, 1):
                            kt = 2 * kth + half
                            nc.tensor.matmul(
                                ps[:, half, :],
                                kT[:, h, kt * P:(kt + 1) * P],
                                qT[:, h, qsl], start=True, stop=True)
                        nc.scalar.activation(
                            expt[:, 2 * kth:2 * kth + 2, :], ps[:],
                            Act.Exp, scale=INV_SQRT_HD)
                    return expt

                def av_unnorm(qc, h, expt):
                    qsl = slice(qc * SC, (qc + 1) * SC)
                    # AV accumulation on PE: fp8 DoubleRow over token-tile
                    # pairs (8 instructions contract all 16 k-tiles).
                    # attn_out is written UNNORMALIZED so pav frees
                    # immediately; the 1/den fix-up happens two iterations
                    # later, giving the GpSimd+DVE denominator tree a full
                    # head of slack off every engine's critical path.
                    pav = psav.tile([P, SC], f32, tag="av", name="av")
                    for kp in range(ST // 2):
                        nc.tensor.matmul(
                            pav[:],
                            v8[:, 2 * kp:2 * kp + 2, h * P:(h + 1) * P],
                            expt[:, 2 * kp:2 * kp + 2, :],
                            start=(kp == 0), stop=(kp == ST // 2 - 1),
                            perf_mode=DR)
                    nc.vector.tensor_copy(attn_outT[:, h, qsl], pav[:])
                    # denominator tree: big fp8 level on GpSimd, f16 tail
                    # on DVE
                    s1 = at_pool.tile([P, 8, SC], f16, tag="s1", name="s1")
                    nc.gpsimd.tensor_tensor(
                        s1[:], expt[:, 0:8, :], expt[:, 8:16, :], Alu.add)
                    s2 = at_pool.tile([P, 4, SC], f16, tag="s2", name="s2")
                    nc.vector.tensor_tensor(
                        s2[:], s1[:, 0:4, :], s1[:, 4:8, :], Alu.add)
                    s3 = at_pool.tile([P, 2, SC], f16, tag="s3", name="s3")
                    nc.vector.tensor_tensor(
                        s3[:], s2[:, 0:2, :], s2[:, 2:4, :], Alu.add)
                    acc = at_pool.tile([P, SC], f16, tag="acc", name="acc")
                    nc.vector.tensor_add(acc[:], s3[:, 0, :], s3[:, 1, :])
                    return acc

                def norm_fix(qc, h, acc):
                    qsl = slice(qc * SC, (qc + 1) * SC)
                    # 128-way partition reduce of the per-partition sums,
                    # broadcast to all partitions; then normalize in place.
                    psm = pssm.tile([P, SC], f32, tag="sum", name="sum")
                    nc.tensor.matmul(
                        psm[:], ones16[:], acc[:], start=True, stop=True)
                    rec_bc = at_pool.tile([P, SC], f32, tag="rec_bc",
                                          name="rec")
                    nc.vector.reciprocal_approx_fast(rec_bc[:], psm[:])
                    nc.vector.tensor_tensor(
                        attn_outT[:, h, qsl], attn_outT[:, h, qsl],
                        rec_bc[:], Alu.mult)

                def o_chains(qc, mts):
                    qsl = slice(qc * SC, (qc + 1) * SC)
                    for mt in mts:
                        msl = slice(mt * P, (mt + 1) * P)
                        ps = pso.tile([P, SC], f32, tag="opsum",
                                      name="opsum")
                        for ct in range(NHG):
                            nc.tensor.matmul(
                                ps[:], wo_t[:, ct, msl],
                                attn_outT[:, ct, qsl],
                                start=(ct == 0), stop=(ct == NHG - 1))
                        ot = ot_pool.tile([P, SC], f16, tag="ot", name="ot")
                        nc.vector.tensor_copy(ot[:], ps[:])
                        nc.sync.dma_start(outT[msl, qsl], ot[:])

                # software pipeline: AV(h) runs on PE behind scores(h+1),
                # so PE never waits on the exp tail of its own head. The
                # o_proj of a finished chunk is emitted in 4-chain slices
                # interleaved between heads - one contiguous 64-matmul
                # o_proj block would starve ACT of fresh score psums for
                # ~15us (PE executes in program order).
                prev = None    # awaiting av_unnorm
                prev2 = None   # awaiting norm_fix
                pending_o = []   # (qc, [mt...]) quarters ready to emit
                for qc in range(NSC):
                    for h in range(NHG):
                        expt = scores_exp(qc, h)
                        if prev is not None:
                            acc = av_unnorm(*prev)
                            nprev2 = (prev[0], prev[1], acc)
                        else:
                            nprev2 = None
                        if prev2 is not None:
                            norm_fix(*prev2)
                            if prev2[1] == NHG - 1:
                                oqc = prev2[0]
                                for quarter in range(4):
                                    pending_o.append(
                                        (oqc, range(quarter * 4,
                                                    quarter * 4 + 4)))
                        if pending_o:
                            o_chains(*pending_o.pop(0))
                        prev2 = nprev2
                        prev = (qc, h, expt)
                acc = av_unnorm(*prev)
                if prev2 is not None:
                    norm_fix(*prev2)
                norm_fix(prev[0], prev[1], acc)
                for qcq, mts in pending_o:
                    o_chains(qcq, mts)
                o_chains(NSC - 1, range(H // P))

    nc.compile()
    return nc


# ---------------------------------------------------------------------------
# Host side: shard inputs, run SPMD, gather.
# ---------------------------------------------------------------------------

def _rope_cos_sin(seq_len, dim, base=10000.0):
    inv_freq = 1.0 / (base ** (np.arange(0, dim, 2, dtype=np.float32) / dim))
    t = np.arange(seq_len, dtype=np.float32)
    freqs = np.outer(t, inv_freq).astype(np.float32)
    emb = np.concatenate([freqs, freqs], -1)
    return np.cos(emb).astype(np.float32), np.sin(emb).astype(np.float32)


def _to_f8(x, scale):
    return np.clip(np.asarray(x, np.float32) * scale, -240.0, 240.0).astype(F8)


def _sw(x, nt):
    """[nt*P, F] -> partition-major [P, nt, F] (long contiguous rows)."""
    x = np.asarray(x)
    return np.ascontiguousarray(
        x.reshape(nt, P, x.shape[-1]).transpose(1, 0, 2))


def make_in_maps(hidden_states, Wq_down, bq_down, Wkv_down, bkv_down,
                 Wq_up, bq_up, Wk_up, bk_up, Wv_up, bv_up, Wo, bo,
                 kv_fp8=KV_FP8):
    cos, sin = _rope_cos_sin(S, ROPE_DIM)
    WkvdT = np.ascontiguousarray(Wkv_down.T)
    # [H, LAT] -> m-major [LT, P, HT, P]
    Wkvd8_sw = np.ascontiguousarray(
        _to_f8(WkvdT, S_WKVD).reshape(H // P, P, LAT // P, P)
        .transpose(2, 1, 0, 3))
    # [H, S] -> chunk-major [NSC, P, HT, SC]
    hs8 = [np.ascontiguousarray(
        _to_f8(np.ascontiguousarray(hidden_states[b].T), S_HS)
        .reshape(H // P, P, S // SC, SC).transpose(2, 1, 0, 3))
        for b in range(B)]
    in_maps = []
    for c in range(8):
        b, j = c // 4, c % 4
        heads = [j, 4 + j, 8 + j, 12 + j]
        x1 = slice(j * P, (j + 1) * P)
        x2 = slice(512 + j * P, 512 + (j + 1) * P)
        vrows = np.concatenate(
            [np.arange(h * P, (h + 1) * P) for h in heads])
        # fused q path: q_half_sel = hs @ (Wqd.T @ Wqu_sel.T) + b_eff
        Wqu_sel = np.concatenate([Wq_up[x1], Wq_up[x2]], 0)  # [256, LAT]
        Wqe = (Wqu_sel.astype(np.float64)
               @ Wq_down.astype(np.float64))                 # [256, H]
        bqe = (Wqu_sel.astype(np.float64) @ bq_down.astype(np.float64)
               + np.concatenate([bq_up[x1], bq_up[x2]]).astype(np.float64))
        im = dict(
            hs8=hs8[b],
            Wkvd8=Wkvd8_sw,
            bkvd=np.ascontiguousarray(bkv_down),
            Wqe8=np.ascontiguousarray(
                _to_f8(np.ascontiguousarray(Wqe.T), S_WQE)
                .reshape(H // P, P, 2, P).transpose(2, 1, 0, 3)),
            Wku=_sw(np.ascontiguousarray(
                np.concatenate([Wk_up[x1], Wk_up[x2]], 0).T).astype(F16),
                LAT // P),
            bqku=np.stack(
                [bqe[:P].astype(np.float32), bqe[P:].astype(np.float32),
                 bk_up[x1], bk_up[x2]], axis=1).copy(),
            Wvu=_sw(np.ascontiguousarray(
                Wv_up[vrows].T).astype(F16), LAT // P),
            bvu=np.ascontiguousarray(bv_up[vrows][None, :]),
            Wo=_sw(np.ascontiguousarray(
                Wo[:, vrows].T).astype(F16), NHG),
            cosT=np.ascontiguousarray(cos[:, x1].T).astype(F16),
            sinT=np.ascontiguousarray(sin[:, x1].T).astype(F16),
        )
        in_maps.append(im)
    return in_maps


_NC_CACHE = {}


def _get_nc():
    if "nc" not in _NC_CACHE:
        _NC_CACHE["nc"] = build_mla()
    return _NC_CACHE["nc"]


LAST_RESULTS = None  # BassKernelResults of the most recent kernel() call


def kernel(**inputs):
    global LAST_RESULTS
    nc = _get_nc()
    in_maps = make_in_maps(**inputs)
    trace = bool(int(os.environ.get("MLA_TRACE", "0")))
    kwargs = {}
    if trace:
        tc_env = os.environ.get("MLA_TRACE_CORES", "0,1,2,3,4,5,6,7")
        kwargs["trace_cores"] = [int(x) for x in tc_env.split(",")]
    res = run_bass_kernel_spmd(
        nc, in_maps, core_ids=list(range(8)), trace=trace, **kwargs)
    LAST_RESULTS = res
    bo = inputs["bo"]
    out = np.zeros((B, S, H), np.float32)
    for b in range(B):
        acc = res.results[b * 4]["outT"].astype(np.float32)
        for j in range(1, 4):
            acc = acc + res.results[b * 4 + j]["outT"]
        out[b] = acc.T + bo[None, :]
    return out


# revision 40
# speedup vs baseline: 1.2902x; 1.2902x over previous
"""Trainium2 Bass kernel for MultiHeadLatentAttention (B=2, S=2048, H=2048,
NH=16, HD=128, LAT=512), SPMD across 8 NeuronCores.

Sharding: 8 cores = 2 (batch) x 4 (head-group TP). Core c handles batch c//4
and head group j = c%4 = heads {j, 4+j, 8+j, 12+j}. That grouping is chosen so
the 4 heads share exactly 256 rows of Wq_up/Wk_up: heads j and 4+j are the raw
x1/x2 slices of q_half, heads 8+j and 12+j are their RoPE combinations - so
the up-projection shards 4-way with no duplication. Each core computes its
partial o_proj output; the host sums the 4 partials per batch and adds bo.

v2: fp8(e4m3) DoubleRow matmuls (2 k-tiles per instruction = 2x PE rate) for
the fused-q projection and the attention AV contraction; exp probabilities are
written as fp8 by the ACT exp; softmax denominator is a 4-instruction balanced
strided tree on DVE. kv_down optionally fp8 via KV_FP8.

Self-contained: builds + compiles the Bass program on first call (cached),
runs via run_bass_kernel_spmd on cores 0-7.
"""
import os
import sys
import types
from contextlib import ExitStack

import numpy as np

if "/opt/trn_rl_repo" not in sys.path:
    sys.path.insert(0, "/opt/trn_rl_repo")

import ml_dtypes

# ---------------------------------------------------------------------------
# NTFF-profile shim: antenv.axon_hooks is missing in this image; register a
# hook backed by the axon PJRT .so so trace=True can capture HW exec time.
# ---------------------------------------------------------------------------


def _install_axon_hooks_shim():
    if "antenv.axon_hooks" in sys.modules:
        return
    try:
        import antenv
        from trn_agent_boot.trn_boot import _ntff_profile_via_ctypes
        hook = _ntff_profile_via_ctypes("/opt/axon/libaxon_pjrt.so")
    except Exception:
        return
    mod = types.ModuleType("antenv.axon_hooks")
    mod.get_axon_ntff_profile_hook = lambda: hook
    mod.set_axon_ntff_profile_hook = lambda h: None
    sys.modules["antenv.axon_hooks"] = mod
    antenv.axon_hooks = mod


_install_axon_hooks_shim()

import concourse.bass as bass  # noqa: E402
import concourse.mybir as mybir  # noqa: E402
import concourse.tile as tile  # noqa: E402
from concourse import bacc  # noqa: E402
from concourse.bass_utils import run_bass_kernel_spmd  # noqa: E402

P = 128
H = 2048
NH = 16
HD = 128
LAT = 512
B = 2
S = 2048
ROPE_DIM = H // 4
NHG = 4          # heads per core
SC = 512         # s/q chunk (one PSUM bank of fp32)
INV_SQRT_HD = 0.08838834764831845  # 1/sqrt(128)

# fp8 pre-scales (host side, folded back out via ACT scale)
S_HS = 32.0       # hidden_states: absmax ~5.3 -> 170 < 240
S_WQE = 1024.0    # fused q weight: absmax ~0.05 -> 51
S_WKVD = 512.0    # kv_down weight: absmax ~0.1 -> 51
KV_FP8 = bool(int(os.environ.get("MLA_KV_FP8", "1")))

f32 = mybir.dt.float32
f32r = mybir.dt.float32r
bf16 = mybir.dt.bfloat16
f16 = mybir.dt.float16
f8 = mybir.dt.float8e4
Act = mybir.ActivationFunctionType
Alu = mybir.AluOpType
DR = mybir.MatmulPerfMode.DoubleRow
BF16 = ml_dtypes.bfloat16
F8 = ml_dtypes.float8_e4m3
F16 = np.float16


def build_mla(seq=S, kv_fp8=KV_FP8, debug=False):
    """Build one core's program. All cores run this same program SPMD."""
    NSC = seq // SC   # s-chunks
    HT = H // P       # 16 h-tiles
    LT = LAT // P     # 4 l-tiles
    ST = seq // P     # s-tiles (= k-tiles in attention)

    nc = bacc.Bacc("TRN2", target_bir_lowering=False, debug=debug)

    # all inputs host-swizzled to partition-major [P, nt, free] so each is
    # one DMA of long contiguous per-partition rows (the naive [H, S]
    # layouts load as 512B descriptors and are descriptor-rate-bound)
    hs8 = nc.dram_tensor("hs8", [seq // SC, P, H // P, SC], f8,
                         kind="ExternalInput")
    assert kv_fp8, "f16 kv_down path removed in v3.3"
    Wkvd8 = nc.dram_tensor("Wkvd8", [LAT // P, P, H // P, P], f8,
                           kind="ExternalInput")
    Wqe8 = nc.dram_tensor("Wqe8", [2, P, H // P, P], f8,
                          kind="ExternalInput")
    bkvd = nc.dram_tensor("bkvd", [LAT], f32, kind="ExternalInput")
    Wku = nc.dram_tensor("Wku", [P, LAT // P, 2 * P], f16,
                         kind="ExternalInput")
    bqku = nc.dram_tensor("bqku", [P, 4], f32, kind="ExternalInput")
    Wvu = nc.dram_tensor("Wvu", [P, LAT // P, NHG * P], f16,
                         kind="ExternalInput")
    bvu = nc.dram_tensor("bvu", [1, NHG * P], f32, kind="ExternalInput")
    Wo = nc.dram_tensor("Wo", [P, NHG, H], f16, kind="ExternalInput")
    cosT = nc.dram_tensor("cosT", [P, seq], f16, kind="ExternalInput")
    sinT = nc.dram_tensor("sinT", [P, seq], f16, kind="ExternalInput")
    outT = nc.dram_tensor("outT", [H, seq], f16, kind="ExternalOutput")

    with tile.TileContext(nc) as tc, ExitStack() as top:
        const = top.enter_context(tc.tile_pool(name="const", bufs=1))
        ao_pool = top.enter_context(tc.tile_pool(name="ao", bufs=1))

        bkvd_t = const.tile([P, LT], f32)
        nc.sync.dma_start(bkvd_t[:], bkvd.rearrange("(o p) -> p o", p=P))
        # on-chip ones (fp8 so it can be the DoubleRow stationary of the
        # softmax-denominator reduction): no DMA dependency, so the HAM
        # warmup starts as soon as the DVE is live
        ones8 = const.tile([P, 2, P], f8)
        nc.vector.memset(ones8[:], 1.0)

        # HAM warmup: back-to-back matmuls (~3.5us of PE activity) while
        # the initial weight/activation DMAs stream in, so the first real
        # matmuls run at 2.4GHz instead of the cold 1.2GHz.
        with tc.tile_pool(name="warm", bufs=1, space="PSUM") as warm_pool:
            wtiles = [warm_pool.tile([P, P], f32, tag=f"w{i}", name=f"warm{i}")
                      for i in range(4)]
            for i in range(96):
                nc.tensor.matmul(wtiles[i % 4][:], ones8[:, 0, :],
                                 ones8[:, 1, :], start=True, stop=True)

        attn_outT = ao_pool.tile([P, NHG, seq], f16)

        with ExitStack() as qkv_scope:
            qk_pool = qkv_scope.enter_context(tc.tile_pool(name="qk", bufs=1))
            v_pool = qkv_scope.enter_context(tc.tile_pool(name="v", bufs=1))
            qT = qk_pool.tile([P, NHG, seq], f16)  # 0=x1, 1=x2, 2,3=rope
            kT = qk_pool.tile([P, NHG, seq], f16)
            v8 = v_pool.tile([P, ST, NHG * P], f8)  # token-major v, fp8

            with ExitStack() as lat_scope:
                lat_pool = lat_scope.enter_context(
                    tc.tile_pool(name="lat", bufs=1))
                kv_latT = lat_pool.tile([P, LT, seq], f16)
                # early-U pool + U psum hoisted ABOVE the D pools: v-up can
                # start the moment the last D chain drains, instead of
                # waiting for D's pools to release and ~2.5MB of U DMAs.
                ue_pool = lat_scope.enter_context(
                    tc.tile_pool(name="uearly", bufs=1))
                psu = lat_scope.enter_context(
                    tc.tile_pool(name="psu", bufs=4, space="PSUM"))
                bqku_t = ue_pool.tile([P, 4], f32)
                bvu_bc = ue_pool.tile([P, NHG * P], f32)
                wvu_t = ue_pool.tile([P, LT, NHG * P], f16)
                cos_t = ue_pool.tile([P, seq], f16)
                sin_t = ue_pool.tile([P, seq], f16)
                wku_t = ue_pool.tile([P, LT, 2 * P], f16)
                ut_pool = lat_scope.enter_context(
                    tc.tile_pool(name="ut", bufs=4))

                # ---------------- phase D: projections from hs -------------
                # per s-chunk: 4 kv_lat chains + 2 fused-q chains. The q
                # down+up pair is algebraically folded into one [H, 256]
                # effective weight (Wqd.T @ Wqu_sel.T) on the host, so the
                # duplicated q-down never runs on-device. The fused-q (and
                # optionally kv) chains run as fp8 DoubleRow: 8 instructions
                # contract all 16 h-tiles.
                with tc.tile_pool(name="wd", bufs=1) as wd_pool, \
                     tc.tile_pool(name="psd", bufs=4, space="PSUM") as psd:
                    # load order: activations first (chains need all 16
                    # h-tiles), then Wkv_down (chains m=0..3), then Wqe
                    h8 = wd_pool.tile([P, HT, seq], f8)
                    wkvd8_t = wd_pool.tile([P, HT, LAT], f8)
                    wqe8_t = wd_pool.tile([P, HT, 2 * P], f8)
                    # issue order = first-chain critical path first: chunk 0
                    # activations + the Wkv_down quarters, then everything
                    # else (chunks 1-3 are only needed ~12us later)
                    nc.sync.dma_start(h8[:, :, 0:SC], hs8[0])
                    for m in range(LT):
                        nc.sync.dma_start(
                            wkvd8_t[:, :, m * P:(m + 1) * P], Wkvd8[m])
                    for ci in range(2):
                        nc.sync.dma_start(
                            wqe8_t[:, :, ci * P:(ci + 1) * P], Wqe8[ci])
                    for sc in range(1, NSC):
                        nc.sync.dma_start(
                            h8[:, :, sc * SC:(sc + 1) * SC], hs8[sc])
                    nc.sync.dma_start(bqku_t[:], bqku[:])
                    nc.sync.dma_start(
                        bvu_bc[:], bvu[:].to_broadcast((P, NHG * P)))
                    nc.sync.dma_start(wvu_t[:], Wvu[:])
                    nc.sync.dma_start(wku_t[:], Wku[:])
                    nc.sync.dma_start(cos_t[:], cosT[:])
                    nc.sync.dma_start(sin_t[:], sinT[:])

                    def rope(dstT, ssl):
                        # rope: slot2 = x1*cos - x2*sin,
                        #       slot3 = x1*sin + x2*cos
                        x1 = dstT[:, 0, ssl]
                        x2 = dstT[:, 1, ssl]
                        t1 = ut_pool.tile([P, SC], f16, tag="ropetmp")
                        t2 = ut_pool.tile([P, SC], f16, tag="ropetmp")
                        nc.vector.tensor_mul(t1[:], x1, cos_t[:, ssl])
                        nc.vector.tensor_mul(t2[:], x2, sin_t[:, ssl])
                        nc.vector.tensor_sub(dstT[:, 2, ssl], t1[:], t2[:])
                        t3 = ut_pool.tile([P, SC], f16, tag="ropetmp")
                        t4 = ut_pool.tile([P, SC], f16, tag="ropetmp")
                        nc.vector.tensor_mul(t3[:], x1, sin_t[:, ssl])
                        nc.vector.tensor_mul(t4[:], x2, cos_t[:, ssl])
                        nc.vector.tensor_add(dstT[:, 3, ssl], t3[:], t4[:])

                    # emission order per chunk: kv chains, k_up, k-rope,
                    # q chains, q-rope, v_up - every DVE op lands with PE
                    # matmul work after it, so the k/q tensors a chunk
                    # contributes are ready the moment its PE work drains
                    # (no DVE tail at the D->attention boundary).
                    for sc in range(NSC):
                        ssl = slice(sc * SC, (sc + 1) * SC)
                        for m in range(LT):
                            ps = psd.tile([P, SC], f32)
                            w8 = wkvd8_t[:, :, m * P:(m + 1) * P]
                            for t in range(HT // 2):
                                nc.tensor.matmul(
                                    ps[:], w8[:, 2 * t:2 * t + 2, :],
                                    h8[:, 2 * t:2 * t + 2, ssl],
                                    start=(t == 0),
                                    stop=(t == HT // 2 - 1),
                                    perf_mode=DR)
                            nc.scalar.activation(
                                kv_latT[:, m, ssl], ps[:], Act.Identity,
                                bias=bkvd_t[:, m:m + 1],
                                scale=1.0 / (S_HS * S_WKVD))

                        for ci in (2, 3):  # k_x1, k_x2
                            csl = slice((ci % 2) * P, (ci % 2) * P + P)
                            ps = psu.tile([P, SC], f32, tag="psu")
                            for lt in range(LT):
                                nc.tensor.matmul(
                                    ps[:], wku_t[:, lt, csl],
                                    kv_latT[:, lt, ssl],
                                    start=(lt == 0), stop=(lt == LT - 1))
                            # bias-add on DVE (free-dim broadcast of [P,1])
                            # to keep ACT free for the attention exps
                            nc.vector.tensor_tensor(
                                kT[:, ci % 2, ssl], ps[:],
                                bqku_t[:, ci:ci + 1].to_broadcast((P, SC)),
                                Alu.add)
                        rope(kT, ssl)

                        for ci in range(2):
                            ps = psd.tile([P, SC], f32)
                            wq8 = wqe8_t[:, :, ci * P:(ci + 1) * P]
                            for t in range(HT // 2):
                                nc.tensor.matmul(
                                    ps[:], wq8[:, 2 * t:2 * t + 2, :],
                                    h8[:, 2 * t:2 * t + 2, ssl],
                                    start=(t == 0),
                                    stop=(t == HT // 2 - 1),
                                    perf_mode=DR)
                            # fused q: bias col ci of bqku = b_eff slice
                            nc.scalar.activation(
                                qT[:, ci, ssl], ps[:], Act.Identity,
                                bias=bqku_t[:, ci:ci + 1],
                                scale=1.0 / (S_HS * S_WQE))
                        rope(qT, ssl)

                        for stl in range(SC // P):
                            st = sc * (SC // P) + stl
                            ps = psu.tile([P, NHG * P], f32, tag="psu")
                            for lt in range(LT):
                                nc.tensor.matmul(
                                    ps[:],
                                    kv_latT[:, lt, st * P:(st + 1) * P],
                                    wvu_t[:, lt, :],
                                    start=(lt == 0), stop=(lt == LT - 1))
                            nc.vector.tensor_tensor(
                                v8[:, st, :], ps[:], bvu_bc[:], Alu.add)

            # ------- phase A+O: attention with o_proj interleaved --------
            # qc-outer: once all 4 heads of a q-chunk are normalized, that
            # chunk's o_proj runs on PE underneath the next chunk's
            # ACT-bound score/exp pipeline.
            with tc.tile_pool(name="exp", bufs=3) as exp_pool, \
                 tc.tile_pool(name="wo", bufs=1) as wo_pool, \
                 tc.tile_pool(name="pss", bufs=2, space="PSUM") as pss, \
                 tc.tile_pool(name="psav", bufs=1, space="PSUM") as psav, \
                 tc.tile_pool(name="pssm", bufs=1, space="PSUM") as pssm, \
                 tc.tile_pool(name="pso", bufs=2, space="PSUM") as pso, \
                 tc.tile_pool(name="att", bufs=3) as at_pool, \
                 tc.tile_pool(name="ot", bufs=4) as ot_pool:
                # o_proj weights resident: [c-part, ct, m] layout
                wo_t = wo_pool.tile([P, NHG, H], f16)
                nc.sync.dma_start(wo_t[:], Wo[:])

                def scores_exp(qc, h):
                    qsl = slice(qc * SC, (qc + 1) * SC)
                    expt = exp_pool.tile([P, ST, SC], f8, tag="expt",
                                         name="expt")
                    # k-tiles in pairs: two matmuls fill a 2-bank psum
                    # tile, one ACT exp covers both (amortizes the ~240ns
                    # per-ACT-instruction overhead) and writes fp8 probs.
                    for kth in range(ST // 2):
                        ps = pss.tile([P, 2, SC], f32, tag="score",
                                      name="score")
                        for half in (0, 1):
                            kt = 2 * kth + half
                            nc.tensor.matmul(
                                ps[:, half, :],
                                kT[:, h, kt * P:(kt + 1) * P],
                                qT[:, h, qsl], start=True, stop=True)
                        nc.scalar.activation(
                            expt[:, 2 * kth:2 * kth + 2, :], ps[:],
                            Act.Exp, scale=INV_SQRT_HD)
                    return expt

                def av_norm(qc, h, expt):
                    qsl = slice(qc * SC, (qc + 1) * SC)
                    # AV accumulation on PE: fp8 DoubleRow over token-tile
                    # pairs (8 instructions contract all 16 k-tiles)
                    pav = psav.tile([P, SC], f32, tag="av", name="av")
                    for kp in range(ST // 2):
                        nc.tensor.matmul(
                            pav[:],
                            v8[:, 2 * kp:2 * kp + 2, h * P:(h + 1) * P],
                            expt[:, 2 * kp:2 * kp + 2, :],
                            start=(kp == 0), stop=(kp == ST // 2 - 1),
                            perf_mode=DR)
                    # softmax denominator: all-ones fp8 DoubleRow matmuls
                    # accumulate sum-over-k into psm, already broadcast to
                    # all 128 partitions. PE is the only engine that
                    # reduces fp8 cheaply (DVE/GpSimd read fp8 at ~1
                    # elem/cycle - a reduction tree there put a ~12us
                    # serial chain on the per-head critical path). Runs a
                    # head behind the exps, so no ACT dependency stalls.
                    psm = pssm.tile([P, SC], f32, tag="sum", name="sum")
                    for kp in range(ST // 2):
                        nc.tensor.matmul(
                            psm[:], ones8[:],
                            expt[:, 2 * kp:2 * kp + 2, :],
                            start=(kp == 0), stop=(kp == ST // 2 - 1),
                            perf_mode=DR)
                    rec_bc = at_pool.tile([P, SC], f32, tag="rec_bc",
                                          name="rec")
                    nc.vector.reciprocal_approx_fast(rec_bc[:], psm[:])
                    nc.vector.tensor_tensor(
                        attn_outT[:, h, qsl], pav[:], rec_bc[:], Alu.mult)

                def o_chains(qc, mts):
                    qsl = slice(qc * SC, (qc + 1) * SC)
                    for mt in mts:
                        msl = slice(mt * P, (mt + 1) * P)
                        ps = pso.tile([P, SC], f32, tag="opsum",
                                      name="opsum")
                        for ct in range(NHG):
                            nc.tensor.matmul(
                                ps[:], wo_t[:, ct, msl],
                                attn_outT[:, ct, qsl],
                                start=(ct == 0), stop=(ct == NHG - 1))
                        ot = ot_pool.tile([P, SC], f16, tag="ot", name="ot")
                        nc.vector.tensor_copy(ot[:], ps[:])
                        nc.sync.dma_start(outT[msl, qsl], ot[:])

                # software pipeline: AV(h) runs on PE behind scores(h+1),
                # so PE never waits on the exp tail of its own head. The
                # o_proj of a finished chunk is emitted in 4-chain slices
                # interleaved between heads - one contiguous 64-matmul
                # o_proj block would starve ACT of fresh score psums for
                # ~15us (PE executes in program order).
                prev = None
                pending_o = []   # (qc, [mt...]) quarters ready to emit
                for qc in range(NSC):
                    for h in range(NHG):
                        expt = scores_exp(qc, h)
                        if prev is not None:
                            av_norm(*prev)
                            if prev[1] == NHG - 1:
                                oqc = prev[0]
                                for quarter in range(4):
                                    pending_o.append(
                                        (oqc, range(quarter * 4,
                                                    quarter * 4 + 4)))
                        if pending_o:
                            o_chains(*pending_o.pop(0))
                        prev = (qc, h, expt)
                av_norm(*prev)
                for qcq, mts in pending_o:
                    o_chains(qcq, mts)
                o_chains(NSC - 1, range(H // P))

    nc.compile()
    return nc


# ---------------------------------------------------------------------------
# Host side: shard inputs, run SPMD, gather.
# ---------------------------------------------------------------------------

def _rope_cos_sin(seq_len, dim, base=10000.0):
    inv_freq = 1.0 / (base ** (np.arange(0, dim, 2, dtype=np.float32) / dim))
    t = np.arange(seq_len, dtype=np.float32)
    freqs = np.outer(t, inv_freq).astype(np.float32)
    emb = np.concatenate([freqs, freqs], -1)
    return np.cos(emb).astype(np.float32), np.sin(emb).astype(np.float32)


def _to_f8(x, scale):
    return np.clip(np.asarray(x, np.float32) * scale, -240.0, 240.0).astype(F8)


def _sw(x, nt):
    """[nt*P, F] -> partition-major [P, nt, F] (long contiguous rows)."""
    x = np.asarray(x)
    return np.ascontiguousarray(
        x.reshape(nt, P, x.shape[-1]).transpose(1, 0, 2))


def make_in_maps(hidden_states, Wq_down, bq_down, Wkv_down, bkv_down,
                 Wq_up, bq_up, Wk_up, bk_up, Wv_up, bv_up, Wo, bo,
                 kv_fp8=KV_FP8):
    cos, sin = _rope_cos_sin(S, ROPE_DIM)
    WkvdT = np.ascontiguousarray(Wkv_down.T)
    # [H, LAT] -> m-major [LT, P, HT, P]
    Wkvd8_sw = np.ascontiguousarray(
        _to_f8(WkvdT, S_WKVD).reshape(H // P, P, LAT // P, P)
        .transpose(2, 1, 0, 3))
    # [H, S] -> chunk-major [NSC, P, HT, SC]
    hs8 = [np.ascontiguousarray(
        _to_f8(np.ascontiguousarray(hidden_states[b].T), S_HS)
        .reshape(H // P, P, S // SC, SC).transpose(2, 1, 0, 3))
        for b in range(B)]
    in_maps = []
    for c in range(8):
        b, j = c // 4, c % 4
        heads = [j, 4 + j, 8 + j, 12 + j]
        x1 = slice(j * P, (j + 1) * P)
        x2 = slice(512 + j * P, 512 + (j + 1) * P)
        vrows = np.concatenate(
            [np.arange(h * P, (h + 1) * P) for h in heads])
        # fused q path: q_half_sel = hs @ (Wqd.T @ Wqu_sel.T) + b_eff
        Wqu_sel = np.concatenate([Wq_up[x1], Wq_up[x2]], 0)  # [256, LAT]
        Wqe = (Wqu_sel.astype(np.float64)
               @ Wq_down.astype(np.float64))                 # [256, H]
        bqe = (Wqu_sel.astype(np.float64) @ bq_down.astype(np.float64)
               + np.concatenate([bq_up[x1], bq_up[x2]]).astype(np.float64))
        im = dict(
            hs8=hs8[b],
            Wkvd8=Wkvd8_sw,
            bkvd=np.ascontiguousarray(bkv_down),
            Wqe8=np.ascontiguousarray(
                _to_f8(np.ascontiguousarray(Wqe.T), S_WQE)
                .reshape(H // P, P, 2, P).transpose(2, 1, 0, 3)),
            Wku=_sw(np.ascontiguousarray(
                np.concatenate([Wk_up[x1], Wk_up[x2]], 0).T).astype(F16),
                LAT // P),
            bqku=np.stack(
                [bqe[:P].astype(np.float32), bqe[P:].astype(np.float32),
                 bk_up[x1], bk_up[x2]], axis=1).copy(),
            Wvu=_sw(np.ascontiguousarray(
                Wv_up[vrows].T).astype(F16), LAT // P),
            bvu=np.ascontiguousarray(bv_up[vrows][None, :]),
            Wo=_sw(np.ascontiguousarray(
                Wo[:, vrows].T).astype(F16), NHG),
            cosT=np.ascontiguousarray(cos[:, x1].T).astype(F16),
            sinT=np.ascontiguousarray(sin[:, x1].T).astype(F16),
        )
        in_maps.append(im)
    return in_maps


_NC_CACHE = {}


def _get_nc():
    if "nc" not in _NC_CACHE:
        _NC_CACHE["nc"] = build_mla()
    return _NC_CACHE["nc"]


LAST_RESULTS = None  # BassKernelResults of the most recent kernel() call


def kernel(**inputs):
    global LAST_RESULTS
    nc = _get_nc()
    in_maps = make_in_maps(**inputs)
    trace = bool(int(os.environ.get("MLA_TRACE", "0")))
    kwargs = {}
    if trace:
        tc_env = os.environ.get("MLA_TRACE_CORES", "0,1,2,3,4,5,6,7")
        kwargs["trace_cores"] = [int(x) for x in tc_env.split(",")]
    res = run_bass_kernel_spmd(
        nc, in_maps, core_ids=list(range(8)), trace=trace, **kwargs)
    LAST_RESULTS = res
    bo = inputs["bo"]
    out = np.zeros((B, S, H), np.float32)
    for b in range(B):
        acc = res.results[b * 4]["outT"].astype(np.float32)
        for j in range(1, 4):
            acc = acc + res.results[b * 4 + j]["outT"]
        out[b] = acc.T + bo[None, :]
    return out


# revision 41
# speedup vs baseline: 1.2947x; 1.0034x over previous
"""Trainium2 Bass kernel for MultiHeadLatentAttention (B=2, S=2048, H=2048,
NH=16, HD=128, LAT=512), SPMD across 8 NeuronCores.

Sharding: 8 cores = 2 (batch) x 4 (head-group TP). Core c handles batch c//4
and head group j = c%4 = heads {j, 4+j, 8+j, 12+j}. That grouping is chosen so
the 4 heads share exactly 256 rows of Wq_up/Wk_up: heads j and 4+j are the raw
x1/x2 slices of q_half, heads 8+j and 12+j are their RoPE combinations - so
the up-projection shards 4-way with no duplication. Each core computes its
partial o_proj output; the host sums the 4 partials per batch and adds bo.

fp8(e4m3) DoubleRow matmuls (2 k-tiles per instruction = 2x PE rate) run the
kv_down, fused-q, AV and softmax-denominator contractions; exp probabilities
are written as fp8 directly by the ACT exp; the denominator is an all-ones
fp8 DoubleRow accumulation on PE (the only engine that reduces fp8 cheaply),
emitted one head behind the exps so it never waits on ACT. o_proj and scores
stay f16 (scores gain nothing from DoubleRow at K=128; fp8 o_proj alone costs
~2.8e-2 rel err). o_proj is emitted in 4-chain slices interleaved between
heads so ACT never starves. All inputs are host-swizzled partition-major so
every load is a few long-row DMAs, ordered first-chain-critical-path first.
Measured ~267us vs the 330us f16 baseline; rel err 1.69e-2 (gate 2e-2).

Self-contained: builds + compiles the Bass program on first call (cached),
runs via run_bass_kernel_spmd on cores 0-7.
"""
import os
import sys
import types
from contextlib import ExitStack

import numpy as np

if "/opt/trn_rl_repo" not in sys.path:
    sys.path.insert(0, "/opt/trn_rl_repo")

import ml_dtypes

# ---------------------------------------------------------------------------
# NTFF-profile shim: antenv.axon_hooks is missing in this image; register a
# hook backed by the axon PJRT .so so trace=True can capture HW exec time.
# ---------------------------------------------------------------------------


def _install_axon_hooks_shim():
    if "antenv.axon_hooks" in sys.modules:
        return
    try:
        import antenv
        from trn_agent_boot.trn_boot import _ntff_profile_via_ctypes
        hook = _ntff_profile_via_ctypes("/opt/axon/libaxon_pjrt.so")
    except Exception:
        return
    mod = types.ModuleType("antenv.axon_hooks")
    mod.get_axon_ntff_profile_hook = lambda: hook
    mod.set_axon_ntff_profile_hook = lambda h: None
    sys.modules["antenv.axon_hooks"] = mod
    antenv.axon_hooks = mod


_install_axon_hooks_shim()

import concourse.bass as bass  # noqa: E402
import concourse.mybir as mybir  # noqa: E402
import concourse.tile as tile  # noqa: E402
from concourse import bacc  # noqa: E402
from concourse.bass_utils import run_bass_kernel_spmd  # noqa: E402

P = 128
H = 2048
NH = 16
HD = 128
LAT = 512
B = 2
S = 2048
ROPE_DIM = H // 4
NHG = 4          # heads per core
SC = 512         # s/q chunk (one PSUM bank of fp32)
INV_SQRT_HD = 0.08838834764831845  # 1/sqrt(128)

# fp8 pre-scales (host side, folded back out via ACT scale)
S_HS = 32.0       # hidden_states: absmax ~5.3 -> 170 < 240
S_WQE = 1024.0    # fused q weight: absmax ~0.05 -> 51
S_WKVD = 512.0    # kv_down weight: absmax ~0.1 -> 51
KV_FP8 = bool(int(os.environ.get("MLA_KV_FP8", "1")))

f32 = mybir.dt.float32
f32r = mybir.dt.float32r
bf16 = mybir.dt.bfloat16
f16 = mybir.dt.float16
f8 = mybir.dt.float8e4
Act = mybir.ActivationFunctionType
Alu = mybir.AluOpType
DR = mybir.MatmulPerfMode.DoubleRow
BF16 = ml_dtypes.bfloat16
F8 = ml_dtypes.float8_e4m3
F16 = np.float16


def build_mla(seq=S, kv_fp8=KV_FP8, debug=False):
    """Build one core's program. All cores run this same program SPMD."""
    NSC = seq // SC   # s-chunks
    HT = H // P       # 16 h-tiles
    LT = LAT // P     # 4 l-tiles
    ST = seq // P     # s-tiles (= k-tiles in attention)

    nc = bacc.Bacc("TRN2", target_bir_lowering=False, debug=debug)

    # all inputs host-swizzled to partition-major [P, nt, free] so each is
    # one DMA of long contiguous per-partition rows (the naive [H, S]
    # layouts load as 512B descriptors and are descriptor-rate-bound)
    hs8 = nc.dram_tensor("hs8", [seq // SC, P, H // P, SC], f8,
                         kind="ExternalInput")
    assert kv_fp8, "f16 kv_down path removed in v3.3"
    Wkvd8 = nc.dram_tensor("Wkvd8", [LAT // P, P, H // P, P], f8,
                           kind="ExternalInput")
    Wqe8 = nc.dram_tensor("Wqe8", [2, P, H // P, P], f8,
                          kind="ExternalInput")
    bkvd = nc.dram_tensor("bkvd", [LAT], f32, kind="ExternalInput")
    Wku = nc.dram_tensor("Wku", [P, LAT // P, 2 * P], f16,
                         kind="ExternalInput")
    bqku = nc.dram_tensor("bqku", [P, 4], f32, kind="ExternalInput")
    Wvu = nc.dram_tensor("Wvu", [P, LAT // P, NHG * P], f16,
                         kind="ExternalInput")
    bvu = nc.dram_tensor("bvu", [1, NHG * P], f32, kind="ExternalInput")
    Wo = nc.dram_tensor("Wo", [P, NHG, H], f16, kind="ExternalInput")
    cosT = nc.dram_tensor("cosT", [P, seq], f16, kind="ExternalInput")
    sinT = nc.dram_tensor("sinT", [P, seq], f16, kind="ExternalInput")
    outT = nc.dram_tensor("outT", [H, seq], f16, kind="ExternalOutput")

    with tile.TileContext(nc) as tc, ExitStack() as top:
        const = top.enter_context(tc.tile_pool(name="const", bufs=1))
        ao_pool = top.enter_context(tc.tile_pool(name="ao", bufs=1))

        bkvd_t = const.tile([P, LT], f32)
        nc.sync.dma_start(bkvd_t[:], bkvd.rearrange("(o p) -> p o", p=P))
        # on-chip ones (fp8 so it can be the DoubleRow stationary of the
        # softmax-denominator reduction): no DMA dependency, so the HAM
        # warmup starts as soon as the DVE is live
        ones8 = const.tile([P, 2, P], f8)
        nc.vector.memset(ones8[:], 1.0)

        # HAM warmup: back-to-back matmuls (~3.5us of PE activity) while
        # the initial weight/activation DMAs stream in, so the first real
        # matmuls run at 2.4GHz instead of the cold 1.2GHz.
        with tc.tile_pool(name="warm", bufs=1, space="PSUM") as warm_pool:
            wtiles = [warm_pool.tile([P, P], f32, tag=f"w{i}", name=f"warm{i}")
                      for i in range(4)]
            for i in range(96):
                nc.tensor.matmul(wtiles[i % 4][:], ones8[:, 0, :],
                                 ones8[:, 1, :], start=True, stop=True)

        attn_outT = ao_pool.tile([P, NHG, seq], f16)

        with ExitStack() as qkv_scope:
            qk_pool = qkv_scope.enter_context(tc.tile_pool(name="qk", bufs=1))
            v_pool = qkv_scope.enter_context(tc.tile_pool(name="v", bufs=1))
            qT = qk_pool.tile([P, NHG, seq], f16)  # 0=x1, 1=x2, 2,3=rope
            kT = qk_pool.tile([P, NHG, seq], f16)
            v8 = v_pool.tile([P, ST, NHG * P], f8)  # token-major v, fp8

            with ExitStack() as lat_scope:
                lat_pool = lat_scope.enter_context(
                    tc.tile_pool(name="lat", bufs=1))
                kv_latT = lat_pool.tile([P, LT, seq], f16)
                # early-U pool + U psum hoisted ABOVE the D pools: v-up can
                # start the moment the last D chain drains, instead of
                # waiting for D's pools to release and ~2.5MB of U DMAs.
                ue_pool = lat_scope.enter_context(
                    tc.tile_pool(name="uearly", bufs=1))
                psu = lat_scope.enter_context(
                    tc.tile_pool(name="psu", bufs=4, space="PSUM"))
                bqku_t = ue_pool.tile([P, 4], f32)
                bvu_bc = ue_pool.tile([P, NHG * P], f32)
                wvu_t = ue_pool.tile([P, LT, NHG * P], f16)
                cos_t = ue_pool.tile([P, seq], f16)
                sin_t = ue_pool.tile([P, seq], f16)
                wku_t = ue_pool.tile([P, LT, 2 * P], f16)
                ut_pool = lat_scope.enter_context(
                    tc.tile_pool(name="ut", bufs=4))

                # ---------------- phase D: projections from hs -------------
                # per s-chunk: 4 kv_lat chains + 2 fused-q chains. The q
                # down+up pair is algebraically folded into one [H, 256]
                # effective weight (Wqd.T @ Wqu_sel.T) on the host, so the
                # duplicated q-down never runs on-device. The fused-q (and
                # optionally kv) chains run as fp8 DoubleRow: 8 instructions
                # contract all 16 h-tiles.
                with tc.tile_pool(name="wd", bufs=1) as wd_pool, \
                     tc.tile_pool(name="psd", bufs=4, space="PSUM") as psd:
                    # load order: activations first (chains need all 16
                    # h-tiles), then Wkv_down (chains m=0..3), then Wqe
                    h8 = wd_pool.tile([P, HT, seq], f8)
                    wkvd8_t = wd_pool.tile([P, HT, LAT], f8)
                    wqe8_t = wd_pool.tile([P, HT, 2 * P], f8)
                    # issue order = first-chain critical path first: chunk 0
                    # activations + the Wkv_down quarters, then everything
                    # else (chunks 1-3 are only needed ~12us later)
                    nc.sync.dma_start(h8[:, :, 0:SC], hs8[0])
                    for m in range(LT):
                        nc.sync.dma_start(
                            wkvd8_t[:, :, m * P:(m + 1) * P], Wkvd8[m])
                    for ci in range(2):
                        nc.sync.dma_start(
                            wqe8_t[:, :, ci * P:(ci + 1) * P], Wqe8[ci])
                    for sc in range(1, NSC):
                        nc.sync.dma_start(
                            h8[:, :, sc * SC:(sc + 1) * SC], hs8[sc])
                    nc.sync.dma_start(bqku_t[:], bqku[:])
                    nc.sync.dma_start(
                        bvu_bc[:], bvu[:].to_broadcast((P, NHG * P)))
                    nc.sync.dma_start(wvu_t[:], Wvu[:])
                    nc.sync.dma_start(wku_t[:], Wku[:])
                    nc.sync.dma_start(cos_t[:], cosT[:])
                    nc.sync.dma_start(sin_t[:], sinT[:])

                    def rope(dstT, ssl):
                        # rope: slot2 = x1*cos - x2*sin,
                        #       slot3 = x1*sin + x2*cos
                        x1 = dstT[:, 0, ssl]
                        x2 = dstT[:, 1, ssl]
                        t1 = ut_pool.tile([P, SC], f16, tag="ropetmp")
                        t2 = ut_pool.tile([P, SC], f16, tag="ropetmp")
                        nc.vector.tensor_mul(t1[:], x1, cos_t[:, ssl])
                        nc.vector.tensor_mul(t2[:], x2, sin_t[:, ssl])
                        nc.vector.tensor_sub(dstT[:, 2, ssl], t1[:], t2[:])
                        t3 = ut_pool.tile([P, SC], f16, tag="ropetmp")
                        t4 = ut_pool.tile([P, SC], f16, tag="ropetmp")
                        nc.vector.tensor_mul(t3[:], x1, sin_t[:, ssl])
                        nc.vector.tensor_mul(t4[:], x2, cos_t[:, ssl])
                        nc.vector.tensor_add(dstT[:, 3, ssl], t3[:], t4[:])

                    # emission order per chunk: kv chains, k_up, k-rope,
                    # q chains, q-rope, v_up - every DVE op lands with PE
                    # matmul work after it, so the k/q tensors a chunk
                    # contributes are ready the moment its PE work drains
                    # (no DVE tail at the D->attention boundary).
                    for sc in range(NSC):
                        ssl = slice(sc * SC, (sc + 1) * SC)
                        for m in range(LT):
                            ps = psd.tile([P, SC], f32)
                            w8 = wkvd8_t[:, :, m * P:(m + 1) * P]
                            for t in range(HT // 2):
                                nc.tensor.matmul(
                                    ps[:], w8[:, 2 * t:2 * t + 2, :],
                                    h8[:, 2 * t:2 * t + 2, ssl],
                                    start=(t == 0),
                                    stop=(t == HT // 2 - 1),
                                    perf_mode=DR)
                            nc.scalar.activation(
                                kv_latT[:, m, ssl], ps[:], Act.Identity,
                                bias=bkvd_t[:, m:m + 1],
                                scale=1.0 / (S_HS * S_WKVD))

                        for ci in (2, 3):  # k_x1, k_x2
                            csl = slice((ci % 2) * P, (ci % 2) * P + P)
                            ps = psu.tile([P, SC], f32, tag="psu")
                            for lt in range(LT):
                                nc.tensor.matmul(
                                    ps[:], wku_t[:, lt, csl],
                                    kv_latT[:, lt, ssl],
                                    start=(lt == 0), stop=(lt == LT - 1))
                            # bias-add on DVE (free-dim broadcast of [P,1])
                            # to keep ACT free for the attention exps
                            nc.vector.tensor_tensor(
                                kT[:, ci % 2, ssl], ps[:],
                                bqku_t[:, ci:ci + 1].to_broadcast((P, SC)),
                                Alu.add)
                        rope(kT, ssl)

                        for ci in range(2):
                            ps = psd.tile([P, SC], f32)
                            wq8 = wqe8_t[:, :, ci * P:(ci + 1) * P]
                            for t in range(HT // 2):
                                nc.tensor.matmul(
                                    ps[:], wq8[:, 2 * t:2 * t + 2, :],
                                    h8[:, 2 * t:2 * t + 2, ssl],
                                    start=(t == 0),
                                    stop=(t == HT // 2 - 1),
                                    perf_mode=DR)
                            # fused q: bias col ci of bqku = b_eff slice
                            nc.scalar.activation(
                                qT[:, ci, ssl], ps[:], Act.Identity,
                                bias=bqku_t[:, ci:ci + 1],
                                scale=1.0 / (S_HS * S_WQE))
                        rope(qT, ssl)

                        for stl in range(SC // P):
                            st = sc * (SC // P) + stl
                            ps = psu.tile([P, NHG * P], f32, tag="psu")
                            for lt in range(LT):
                                nc.tensor.matmul(
                                    ps[:],
                                    kv_latT[:, lt, st * P:(st + 1) * P],
                                    wvu_t[:, lt, :],
                                    start=(lt == 0), stop=(lt == LT - 1))
                            nc.vector.tensor_tensor(
                                v8[:, st, :], ps[:], bvu_bc[:], Alu.add)

            # ------- phase A+O: attention with o_proj interleaved --------
            # qc-outer: once all 4 heads of a q-chunk are normalized, that
            # chunk's o_proj runs on PE underneath the next chunk's
            # ACT-bound score/exp pipeline.
            with tc.tile_pool(name="exp", bufs=3) as exp_pool, \
                 tc.tile_pool(name="wo", bufs=1) as wo_pool, \
                 tc.tile_pool(name="pss", bufs=2, space="PSUM") as pss, \
                 tc.tile_pool(name="psav", bufs=1, space="PSUM") as psav, \
                 tc.tile_pool(name="pssm", bufs=1, space="PSUM") as pssm, \
                 tc.tile_pool(name="pso", bufs=2, space="PSUM") as pso, \
                 tc.tile_pool(name="att", bufs=3) as at_pool, \
                 tc.tile_pool(name="ot", bufs=4) as ot_pool:
                # o_proj weights resident: [c-part, ct, m] layout
                wo_t = wo_pool.tile([P, NHG, H], f16)
                nc.sync.dma_start(wo_t[:], Wo[:])

                def scores_exp(qc, h):
                    qsl = slice(qc * SC, (qc + 1) * SC)
                    expt = exp_pool.tile([P, ST, SC], f8, tag="expt",
                                         name="expt")
                    # k-tiles in pairs: two matmuls fill a 2-bank psum
                    # tile, one ACT exp covers both (amortizes the ~240ns
                    # per-ACT-instruction overhead) and writes fp8 probs.
                    for kth in range(ST // 2):
                        ps = pss.tile([P, 2, SC], f32, tag="score",
                                      name="score")
                        for half in (0, 1):
                            kt = 2 * kth + half
                            nc.tensor.matmul(
                                ps[:, half, :],
                                kT[:, h, kt * P:(kt + 1) * P],
                                qT[:, h, qsl], start=True, stop=True)
                        nc.scalar.activation(
                            expt[:, 2 * kth:2 * kth + 2, :], ps[:],
                            Act.Exp, scale=INV_SQRT_HD)
                    return expt

                def av_norm(qc, h, expt):
                    qsl = slice(qc * SC, (qc + 1) * SC)
                    # AV accumulation on PE: fp8 DoubleRow over token-tile
                    # pairs (8 instructions contract all 16 k-tiles)
                    pav = psav.tile([P, SC], f32, tag="av", name="av")
                    for kp in range(ST // 2):
                        nc.tensor.matmul(
                            pav[:],
                            v8[:, 2 * kp:2 * kp + 2, h * P:(h + 1) * P],
                            expt[:, 2 * kp:2 * kp + 2, :],
                            start=(kp == 0), stop=(kp == ST // 2 - 1),
                            perf_mode=DR)
                    # softmax denominator: all-ones fp8 DoubleRow matmuls
                    # accumulate sum-over-k into psm, already broadcast to
                    # all 128 partitions. PE is the only engine that
                    # reduces fp8 cheaply (DVE/GpSimd read fp8 at ~1
                    # elem/cycle - a reduction tree there put a ~12us
                    # serial chain on the per-head critical path). Runs a
                    # head behind the exps, so no ACT dependency stalls.
                    psm = pssm.tile([P, SC], f32, tag="sum", name="sum")
                    for kp in range(ST // 2):
                        nc.tensor.matmul(
                            psm[:], ones8[:],
                            expt[:, 2 * kp:2 * kp + 2, :],
                            start=(kp == 0), stop=(kp == ST // 2 - 1),
                            perf_mode=DR)
                    rec_bc = at_pool.tile([P, SC], f32, tag="rec_bc",
                                          name="rec")
                    nc.vector.reciprocal_approx_fast(rec_bc[:], psm[:])
                    nc.vector.tensor_tensor(
                        attn_outT[:, h, qsl], pav[:], rec_bc[:], Alu.mult)

                def o_chains(qc, mts):
                    qsl = slice(qc * SC, (qc + 1) * SC)
                    for mt in mts:
                        msl = slice(mt * P, (mt + 1) * P)
                        ps = pso.tile([P, SC], f32, tag="opsum",
                                      name="opsum")
                        for ct in range(NHG):
                            nc.tensor.matmul(
                                ps[:], wo_t[:, ct, msl],
                                attn_outT[:, ct, qsl],
                                start=(ct == 0), stop=(ct == NHG - 1))
                        ot = ot_pool.tile([P, SC], f16, tag="ot", name="ot")
                        nc.vector.tensor_copy(ot[:], ps[:])
                        nc.sync.dma_start(outT[msl, qsl], ot[:])

                # software pipeline: AV(h) runs on PE behind scores(h+1),
                # so PE never waits on the exp tail of its own head. The
                # o_proj of a finished chunk is emitted in 4-chain slices
                # interleaved between heads - one contiguous 64-matmul
                # o_proj block would starve ACT of fresh score psums for
                # ~15us (PE executes in program order).
                prev = None
                pending_o = []   # (qc, [mt...]) quarters ready to emit
                for qc in range(NSC):
                    for h in range(NHG):
                        expt = scores_exp(qc, h)
                        if prev is not None:
                            av_norm(*prev)
                            if prev[1] == NHG - 1:
                                oqc = prev[0]
                                for quarter in range(4):
                                    pending_o.append(
                                        (oqc, range(quarter * 4,
                                                    quarter * 4 + 4)))
                        if pending_o:
                            o_chains(*pending_o.pop(0))
                        prev = (qc, h, expt)
                av_norm(*prev)
                for qcq, mts in pending_o:
                    o_chains(qcq, mts)
                o_chains(NSC - 1, range(H // P))

    nc.compile()
    return nc


# ---------------------------------------------------------------------------
# Host side: shard inputs, run SPMD, gather.
# ---------------------------------------------------------------------------

def _rope_cos_sin(seq_len, dim, base=10000.0):
    inv_freq = 1.0 / (base ** (np.arange(0, dim, 2, dtype=np.float32) / dim))
    t = np.arange(seq_len, dtype=np.float32)
    freqs = np.outer(t, inv_freq).astype(np.float32)
    emb = np.concatenate([freqs, freqs], -1)
    return np.cos(emb).astype(np.float32), np.sin(emb).astype(np.float32)


def _to_f8(x, scale):
    return np.clip(np.asarray(x, np.float32) * scale, -240.0, 240.0).astype(F8)


def _sw(x, nt):
    """[nt*P, F] -> partition-major [P, nt, F] (long contiguous rows)."""
    x = np.asarray(x)
    return np.ascontiguousarray(
        x.reshape(nt, P, x.shape[-1]).transpose(1, 0, 2))


def make_in_maps(hidden_states, Wq_down, bq_down, Wkv_down, bkv_down,
                 Wq_up, bq_up, Wk_up, bk_up, Wv_up, bv_up, Wo, bo,
                 kv_fp8=KV_FP8):
    cos, sin = _rope_cos_sin(S, ROPE_DIM)
    WkvdT = np.ascontiguousarray(Wkv_down.T)
    # [H, LAT] -> m-major [LT, P, HT, P]
    Wkvd8_sw = np.ascontiguousarray(
        _to_f8(WkvdT, S_WKVD).reshape(H // P, P, LAT // P, P)
        .transpose(2, 1, 0, 3))
    # [H, S] -> chunk-major [NSC, P, HT, SC]
    hs8 = [np.ascontiguousarray(
        _to_f8(np.ascontiguousarray(hidden_states[b].T), S_HS)
        .reshape(H // P, P, S // SC, SC).transpose(2, 1, 0, 3))
        for b in range(B)]
    in_maps = []
    for c in range(8):
        b, j = c // 4, c % 4
        heads = [j, 4 + j, 8 + j, 12 + j]
        x1 = slice(j * P, (j + 1) * P)
        x2 = slice(512 + j * P, 512 + (j + 1) * P)
        vrows = np.concatenate(
            [np.arange(h * P, (h + 1) * P) for h in heads])
        # fused q path: q_half_sel = hs @ (Wqd.T @ Wqu_sel.T) + b_eff
        Wqu_sel = np.concatenate([Wq_up[x1], Wq_up[x2]], 0)  # [256, LAT]
        Wqe = (Wqu_sel.astype(np.float64)
               @ Wq_down.astype(np.float64))                 # [256, H]
        bqe = (Wqu_sel.astype(np.float64) @ bq_down.astype(np.float64)
               + np.concatenate([bq_up[x1], bq_up[x2]]).astype(np.float64))
        im = dict(
            hs8=hs8[b],
            Wkvd8=Wkvd8_sw,
            bkvd=np.ascontiguousarray(bkv_down),
            Wqe8=np.ascontiguousarray(
                _to_f8(np.ascontiguousarray(Wqe.T), S_WQE)
                .reshape(H // P, P, 2, P).transpose(2, 1, 0, 3)),
            Wku=_sw(np.ascontiguousarray(
                np.concatenate([Wk_up[x1], Wk_up[x2]], 0).T).astype(F16),
                LAT // P),
            bqku=np.stack(
                [bqe[:P].astype(np.float32), bqe[P:].astype(np.float32),
                 bk_up[x1], bk_up[x2]], axis=1).copy(),
            Wvu=_sw(np.ascontiguousarray(
                Wv_up[vrows].T).astype(F16), LAT // P),
            bvu=np.ascontiguousarray(bv_up[vrows][None, :]),
            Wo=_sw(np.ascontiguousarray(
                Wo[:, vrows].T).astype(F16), NHG),
            cosT=np.ascontiguousarray(cos[:, x1].T).astype(F16),
            sinT=np.ascontiguousarray(sin[:, x1].T).astype(F16),
        )
        in_maps.append(im)
    return in_maps


_NC_CACHE = {}


def _get_nc():
    if "nc" not in _NC_CACHE:
        _NC_CACHE["nc"] = build_mla()
    return _NC_CACHE["nc"]


LAST_RESULTS = None  # BassKernelResults of the most recent kernel() call


def kernel(**inputs):
    global LAST_RESULTS
    nc = _get_nc()
    in_maps = make_in_maps(**inputs)
    trace = bool(int(os.environ.get("MLA_TRACE", "0")))
    kwargs = {}
    if trace:
        tc_env = os.environ.get("MLA_TRACE_CORES", "0,1,2,3,4,5,6,7")
        kwargs["trace_cores"] = [int(x) for x in tc_env.split(",")]
    res = run_bass_kernel_spmd(
        nc, in_maps, core_ids=list(range(8)), trace=trace, **kwargs)
    LAST_RESULTS = res
    bo = inputs["bo"]
    out = np.zeros((B, S, H), np.float32)
    for b in range(B):
        acc = res.results[b * 4]["outT"].astype(np.float32)
        for j in range(1, 4):
            acc = acc + res.results[b * 4 + j]["outT"]
        out[b] = acc.T + bo[None, :]
    return out
